# revision 15
# baseline (speedup 1.0000x reference)
"""GCN layer (dense-adj reference semantics) as a Bass/Tile kernel on 8
Trainium2 NeuronCores.

Strategy: edge-partitioned sparse message passing instead of the dense
16384x16384 adjacency.
  out[i] = dis[i] * ( sum_{e:(i,j)} w_e * dis[j] * x[j] + dis[i]... ) @ W
computed as
  xs = dis (.) x                (node scaling, bf16)
  aggT[f, i] = sum_e w_e xs[dst_e, f]   (gather + selector matmuls on PE)
  out = dis (.) (aggT^T @ W)
with deg[j] = 1 + sum_{e: dst=j} w_e reduced on-device from a host-packed
padded layout. Src nodes are row-sharded across the 8 cores; each core
processes only its own edges (grouped by 128-src groups, 128-edge blocks).
Self loops (the +I term) are injected as synthetic edges with weight 1.
Duplicate (src,dst) edges resolve last-wins, matching XLA scatter-set.
"""
import numpy as np
import ml_dtypes

import concourse.bass as bass
import concourse.bacc as bacc
import concourse.mybir as mybir
from concourse.tile import TileContext
from concourse.bass_utils import run_bass_kernel_spmd

P = 128
NC_CORES = 8
N = 16384
F = 128
NPC = N // NC_CORES    # 2048 src rows per core
G = NPC // P           # 16 src groups per core
NG = N // P            # 128 node groups
CB = 7                 # gather chunk size in 128-edge blocks (desc scratch)


# ----------------------------------------------------------------- host prep
def _dedup_last(src, dst, ew):
    key = src.astype(np.int64) * N + dst.astype(np.int64)
    order = np.argsort(key, kind="stable")
    ks = key[order]
    run_last = np.concatenate([ks[1:] != ks[:-1], [True]])
    keep = order[run_last]
    return src[keep], dst[keep], ew[keep]


def prep(x, edge_index, edge_weight, weight):
    src = np.asarray(edge_index[0]).astype(np.int64)
    dst = np.asarray(edge_index[1]).astype(np.int64)
    ew = np.asarray(edge_weight).astype(np.float32)
    s, d, w = _dedup_last(src, dst, ew)
    # synthetic self edges = the +eye term
    s = np.concatenate([s, np.arange(N, dtype=np.int64)])
    d = np.concatenate([d, np.arange(N, dtype=np.int64)])
    w = np.concatenate([w, np.ones(N, dtype=np.float32)])
    E2 = len(s)

    # deg inputs packed [p = j%128, col = (j//128)*LD + rank], zero padded
    order_d = np.argsort(d, kind="stable")
    d_sorted = d[order_d]
    w_sorted = w[order_d]
    cnt_in = np.bincount(d_sorted, minlength=N)
    LD = int(cnt_in.max())
    starts = np.zeros(N + 1, dtype=np.int64)
    starts[1:] = np.cumsum(cnt_in)
    rank = np.arange(E2) - starts[d_sorted]
    wpad = np.zeros((P, NG * LD), dtype=np.float32)
    wpad[(d_sorted // P).astype(np.int64), (d_sorted % P) * LD + rank] = w_sorted

    core = s // NPC
    per_core = []
    B = 0
    for c in range(NC_CORES):
        m_c = core == c
        sc, dc, wc = s[m_c], d[m_c], w[m_c]
        sl = sc - c * NPC
        g = sl // P
        m = sl % P
        cnt_g = np.bincount(g, minlength=G)
        B = max(B, int(np.ceil(cnt_g.max() / P)))
        per_core.append((g, m, dc, wc, cnt_g))

    LDo = 0
    for c in range(NC_CORES):
        lo, hi = c * NPC, (c + 1) * NPC
        m_own = (d_sorted >= lo) & (d_sorted < hi)
        LDo = max(LDo, int(np.bincount(d_sorted[m_own] - lo, minlength=NPC).max()))

    C2 = B * P // 16
    chunks = []
    off = 0
    while off < B:
        sz = min(CB, B - off)
        chunks.append((off, sz))
        off += sz
    bf = ml_dtypes.bfloat16
    x_f32 = np.ascontiguousarray(np.asarray(x).astype(np.float32))
    wmat_f32 = np.ascontiguousarray(np.asarray(weight).astype(np.float32))
    iota_rep = np.tile(np.arange(P, dtype=np.float32), (P, 1)).astype(bf)

    in_maps = []
    for c in range(NC_CORES):
        g, m, dc, wc, cnt_g = per_core[c]
        order_g = np.argsort(g, kind="stable")
        gg, mm, dd, ww = g[order_g], m[order_g], dc[order_g], wc[order_g]
        starts_g = np.zeros(G + 1, dtype=np.int64)
        starts_g[1:] = np.cumsum(cnt_g)
        rank_g = np.arange(len(gg)) - starts_g[gg]
        b_of = rank_g // P
        p_of = rank_g % P

        idx_flat = np.zeros((G, B * P), dtype=np.int16)
        esrc = np.zeros((P, G * B), dtype=np.float32)
        eww = np.zeros((P, G * B), dtype=np.float32)
        idx_flat[gg, rank_g] = dd.astype(np.int16)
        esrc[p_of, gg * B + b_of] = mm.astype(np.float32)
        eww[p_of, gg * B + b_of] = ww
        # gathers are split into <=CB-block chunks (SWDGE descriptor scratch
        # holds ~1024 16B descriptors). wrap per chunk: element i of a chunk
        # reads idxs[i%16, i//16]; replicate the 16-row pattern x8.
        gidx = np.zeros((P, G * C2), dtype=np.int16)
        for gi in range(G):
            col = gi * C2
            for off, sz in chunks:
                seg = idx_flat[gi, off * P:(off + sz) * P]
                wr = seg.reshape(sz * P // 16, 16).T       # [16, sz*8]
                gidx[:, col:col + sz * 8] = np.tile(wr, (8, 1))
                col += sz * 8

        lo = c * NPC
        m_own = (d_sorted >= lo) & (d_sorted < lo + NPC)
        d_own = d_sorted[m_own] - lo
        w_own = w_sorted[m_own]
        cnt_own = np.bincount(d_own, minlength=NPC)
        st = np.zeros(NPC + 1, dtype=np.int64)
        st[1:] = np.cumsum(cnt_own)
        rk = np.arange(len(d_own)) - st[d_own]
        wpad_own = np.zeros((P, G * LDo), dtype=np.float32)
        wpad_own[(d_own % P).astype(np.int64), (d_own // P) * LDo + rk] = w_own

        in_maps.append(dict(
            x=x_f32.astype(bf),
            wpad=wpad.astype(bf),
            wpad_own=wpad_own.astype(bf),
            gidx=gidx,
            esrc=esrc,
            eww=eww,
            wmat=wmat_f32,
            iota_rep=iota_rep,
        ))
    return in_maps, (B, LD, LDo)


# --------------------------------------------------------------- bass kernel
def build(B, LD, LDo):
    f32 = mybir.dt.float32
    bf16 = mybir.dt.bfloat16
    i16 = mybir.dt.int16
    AF = mybir.ActivationFunctionType
    OP = mybir.AluOpType
    AX = mybir.AxisListType
    C2 = B * P // 16

    nc = bacc.Bacc("TRN2", debug=False, num_swdge_queues=4)

    x_d = nc.dram_tensor("x", [N, F], bf16, kind="ExternalInput")
    wpad_d = nc.dram_tensor("wpad", [P, NG * LD], bf16, kind="ExternalInput")
    wpado_d = nc.dram_tensor("wpad_own", [P, G * LDo], bf16, kind="ExternalInput")
    gidx_d = nc.dram_tensor("gidx", [P, G * C2], i16, kind="ExternalInput")
    esrc_d = nc.dram_tensor("esrc", [P, G * B], f32, kind="ExternalInput")
    eww_d = nc.dram_tensor("eww", [P, G * B], f32, kind="ExternalInput")
    wmat_d = nc.dram_tensor("wmat", [F, F], f32, kind="ExternalInput")
    iota_d = nc.dram_tensor("iota_rep", [P, P], bf16, kind="ExternalInput")
    out_d = nc.dram_tensor("out", [NPC, F], f32, kind="ExternalOutput")

    with TileContext(nc) as tc:
        with (
            tc.tile_pool(name="const", bufs=1) as cpool,
            tc.tile_pool(name="pha", bufs=3) as apool,
            tc.tile_pool(name="phb", bufs=3) as bpool,
            tc.tile_pool(name="psum", bufs=2, space="PSUM") as ppool,
            tc.tile_pool(name="psum2", bufs=2, space="PSUM") as ppool2,
            tc.tile_pool(name="dram", bufs=1, space="DRAM") as dpool,
        ):
            from concourse import library_config
            nc.gpsimd.load_library(library_config.mlp)
            # ---- constants ----
            wmat_f = cpool.tile([F, F], f32)
            nc.sync.dma_start(out=wmat_f[:, :], in_=wmat_d[:, :])
            wmat_b = cpool.tile([F, F], bf16)
            nc.vector.tensor_copy(out=wmat_b[:, :], in_=wmat_f[:, :])
            iota_t = cpool.tile([P, P], bf16)
            nc.sync.dma_start(out=iota_t[:, :], in_=iota_d[:, :])
            gidx_t = cpool.tile([P, G * C2], i16)
            nc.sync.dma_start(out=gidx_t[:, :], in_=gidx_d[:, :])
            esrc_t = cpool.tile([P, G * B], f32)
            nc.sync.dma_start(out=esrc_t[:, :], in_=esrc_d[:, :])
            eww_t = cpool.tile([P, G * B], f32)
            nc.sync.dma_start(out=eww_t[:, :], in_=eww_d[:, :])

            # ---- phase A: deg -> dis; xs = dis*x -> DRAM scratch ----
            deg_t = cpool.tile([P, NG], f32)
            NCH = 4
            CG = NG // NCH
            for t in range(NCH):
                wp = apool.tile([P, CG * LD], bf16, tag="wp")
                nc.sync.dma_start(
                    out=wp[:, :], in_=wpad_d[:, t * CG * LD:(t + 1) * CG * LD])
                nc.vector.tensor_reduce(
                    out=deg_t[:, t * CG:(t + 1) * CG],
                    in_=wp[:, :].rearrange("p (g l) -> p g l", l=LD),
                    axis=AX.X, op=OP.add)
            sqd_t = cpool.tile([P, NG], f32)
            nc.scalar.activation(out=sqd_t[:, :], in_=deg_t[:, :], func=AF.Sqrt)
            dis_t = cpool.tile([P, NG], f32)
            nc.vector.reciprocal(out=dis_t[:, :], in_=sqd_t[:, :])
            disb_t = cpool.tile([P, NG], bf16)
            nc.vector.tensor_copy(out=disb_t[:, :], in_=dis_t[:, :])

            dego_t = cpool.tile([P, G], f32)
            wpo = apool.tile([P, G * LDo], bf16, tag="wpo")
            nc.sync.dma_start(out=wpo[:, :], in_=wpado_d[:, :])
            nc.vector.tensor_reduce(
                out=dego_t[:, :],
                in_=wpo[:, :].rearrange("p (g l) -> p g l", l=LDo),
                axis=AX.X, op=OP.add)
            sqdo_t = cpool.tile([P, G], f32)
            nc.scalar.activation(out=sqdo_t[:, :], in_=dego_t[:, :], func=AF.Sqrt)
            diso_t = cpool.tile([P, G], f32)
            nc.vector.reciprocal(out=diso_t[:, :], in_=sqdo_t[:, :])

            xs_dram = dpool.tile([N, F], bf16)
            XCH = 8
            RG = NG // XCH
            x3 = x_d[:, :].rearrange("(p j) f -> p j f", p=P)
            xs3 = xs_dram[:, :].rearrange("(p j) f -> p j f", p=P)
            for t in range(XCH):
                sl = slice(t * RG, (t + 1) * RG)
                xt = apool.tile([P, RG, F], bf16, tag="xt")
                nc.sync.dma_start(out=xt[:, :, :], in_=x3[:, sl, :])
                xst = apool.tile([P, RG, F], bf16, tag="xst")
                nc.vector.tensor_tensor(
                    out=xst[:, :, :],
                    in0=xt[:, :, :],
                    in1=disb_t[:, sl].to_broadcast([P, RG, F]),
                    op=OP.mult)
                nc.sync.dma_start(out=xs3[:, sl, :], in_=xst[:, :, :])

            # ---- phase B: gather + selector matmuls per 128-src group ----
            chunks = []
            off = 0
            while off < B:
                sz = min(CB, B - off)
                chunks.append((off, sz))
                off += sz
            qn = 0
            for g in range(G):
                xg = bpool.tile([P, B, F], bf16, tag="xg")
                for off, sz in chunks:
                    nc.gpsimd.dma_gather(
                        xg[:, off:off + sz, :],
                        xs_dram[:, :],
                        gidx_t[:, g * C2 + off * 8:g * C2 + (off + sz) * 8],
                        sz * P,
                        sz * P,
                        F,
                        elem_step=F,
                        queue_num=qn % 4,
                    )
                    qn += 1
                sel = bpool.tile([P, B * P], bf16, tag="sel")
                for b in range(B):
                    nc.vector.tensor_scalar(
                        out=sel[:, b * P:(b + 1) * P],
                        in0=iota_t[:, :],
                        scalar1=esrc_t[:, g * B + b:g * B + b + 1],
                        scalar2=eww_t[:, g * B + b:g * B + b + 1],
                        op0=OP.is_equal,
                        op1=OP.mult)
                aggp = ppool.tile([F, P], f32, space="PSUM")
                for b in range(B):
                    nc.tensor.matmul(
                        out=aggp[:, :],
                        lhsT=xg[:, b, :],
                        rhs=sel[:, b * P:(b + 1) * P],
                        start=(b == 0),
                        stop=(b == B - 1))
                aggT = bpool.tile([F, P], bf16, tag="aggT")
                nc.vector.tensor_copy(out=aggT[:, :], in_=aggp[:, :])
                outp = ppool2.tile([P, F], f32, space="PSUM")
                nc.tensor.matmul(
                    out=outp[:, :], lhsT=aggT[:, :], rhs=wmat_b[:, :],
                    start=True, stop=True)
                outs = bpool.tile([P, F], f32, tag="outs")
                nc.vector.tensor_scalar(
                    out=outs[:, :], in0=outp[:, :],
                    scalar1=diso_t[:, g:g + 1], scalar2=None, op0=OP.mult)
                nc.sync.dma_start(
                    out=out_d[g * P:(g + 1) * P, :], in_=outs[:, :])

    nc.compile()
    return nc


_NC_CACHE = {}


def run(inputs, trace=False, trace_kwargs=None):
    in_maps, shape_key = prep(
        inputs["x"], inputs["edge_index"], inputs["edge_weight"],
        inputs["weight"])
    if shape_key not in _NC_CACHE:
        _NC_CACHE[shape_key] = build(*shape_key)
    nc = _NC_CACHE[shape_key]
    res = run_bass_kernel_spmd(
        nc, in_maps, core_ids=list(range(NC_CORES)),
        trace=trace, **(trace_kwargs or {}))
    out = np.concatenate([r["out"] for r in res.results], axis=0)
    return out.astype(np.float32), res


def kernel(**inputs):
    out, _ = run(inputs)
    return out


if __name__ == "__main__":
    pass


# revision 16
# speedup vs baseline: 1.2103x; 1.2103x over previous
"""GCN layer (dense-adj reference semantics) as a Bass/Tile kernel on 8
Trainium2 NeuronCores.

Strategy: edge-partitioned sparse message passing instead of the dense
16384x16384 adjacency.
  out[i] = dis[i] * ( sum_{e:(i,j)} w_e * dis[j] * x[j] + dis[i]... ) @ W
computed as
  xs = dis (.) x                (node scaling, bf16)
  aggT[f, i] = sum_e w_e xs[dst_e, f]   (gather + selector matmuls on PE)
  out = dis (.) (aggT^T @ W)
with deg[j] = 1 + sum_{e: dst=j} w_e reduced on-device from a host-packed
padded layout. Src nodes are row-sharded across the 8 cores; each core
processes only its own edges (grouped by 128-src groups, 128-edge blocks).
Self loops (the +I term) are injected as synthetic edges with weight 1.
Duplicate (src,dst) edges resolve last-wins, matching XLA scatter-set.
"""
import numpy as np
import ml_dtypes

import concourse.bass as bass
import concourse.bacc as bacc
import concourse.mybir as mybir
from concourse.tile import TileContext
from concourse.bass_utils import run_bass_kernel_spmd

P = 128
NC_CORES = 8
N = 16384
F = 128
NPC = N // NC_CORES    # 2048 src rows per core
G = NPC // P           # 16 src groups per core
NG = N // P            # 128 node groups
CB = 7                 # gather chunk size in 128-edge blocks (desc scratch)


# ----------------------------------------------------------------- host prep
def _dedup_last(src, dst, ew):
    key = src.astype(np.int64) * N + dst.astype(np.int64)
    order = np.argsort(key, kind="stable")
    ks = key[order]
    run_last = np.concatenate([ks[1:] != ks[:-1], [True]])
    keep = order[run_last]
    return src[keep], dst[keep], ew[keep]


def prep(x, edge_index, edge_weight, weight):
    src = np.asarray(edge_index[0]).astype(np.int64)
    dst = np.asarray(edge_index[1]).astype(np.int64)
    ew = np.asarray(edge_weight).astype(np.float32)
    s, d, w = _dedup_last(src, dst, ew)
    # synthetic self edges = the +eye term
    s = np.concatenate([s, np.arange(N, dtype=np.int64)])
    d = np.concatenate([d, np.arange(N, dtype=np.int64)])
    w = np.concatenate([w, np.ones(N, dtype=np.float32)])
    E2 = len(s)

    # deg inputs packed [p = j%128, col = (j//128)*LD + rank], zero padded
    order_d = np.argsort(d, kind="stable")
    d_sorted = d[order_d]
    w_sorted = w[order_d]
    cnt_in = np.bincount(d_sorted, minlength=N)
    LD = int(cnt_in.max())
    starts = np.zeros(N + 1, dtype=np.int64)
    starts[1:] = np.cumsum(cnt_in)
    rank = np.arange(E2) - starts[d_sorted]
    wpad = np.zeros((P, NG * LD), dtype=np.float32)
    wpad[(d_sorted // P).astype(np.int64), (d_sorted % P) * LD + rank] = w_sorted

    core = s // NPC
    per_core = []
    B = 0
    for c in range(NC_CORES):
        m_c = core == c
        sc, dc, wc = s[m_c], d[m_c], w[m_c]
        sl = sc - c * NPC
        g = sl // P
        m = sl % P
        cnt_g = np.bincount(g, minlength=G)
        B = max(B, int(np.ceil(cnt_g.max() / P)))
        per_core.append((g, m, dc, wc, cnt_g))

    LDo = 0
    for c in range(NC_CORES):
        lo, hi = c * NPC, (c + 1) * NPC
        m_own = (d_sorted >= lo) & (d_sorted < hi)
        LDo = max(LDo, int(np.bincount(d_sorted[m_own] - lo, minlength=NPC).max()))

    C2 = B * P // 16
    chunks = []
    off = 0
    while off < B:
        sz = min(CB, B - off)
        chunks.append((off, sz))
        off += sz
    bf = ml_dtypes.bfloat16
    x_f32 = np.ascontiguousarray(np.asarray(x).astype(np.float32))
    wmat_f32 = np.ascontiguousarray(np.asarray(weight).astype(np.float32))
    iota_rep = np.tile(np.arange(P, dtype=np.float32), (P, B)).astype(bf)

    in_maps = []
    for c in range(NC_CORES):
        g, m, dc, wc, cnt_g = per_core[c]
        order_g = np.argsort(g, kind="stable")
        gg, mm, dd, ww = g[order_g], m[order_g], dc[order_g], wc[order_g]
        starts_g = np.zeros(G + 1, dtype=np.int64)
        starts_g[1:] = np.cumsum(cnt_g)
        rank_g = np.arange(len(gg)) - starts_g[gg]
        b_of = rank_g // P
        p_of = rank_g % P

        idx_flat = np.zeros((G, B * P), dtype=np.int16)
        esrc = np.zeros((P, G * B), dtype=np.float32)
        eww = np.zeros((P, G * B), dtype=np.float32)
        idx_flat[gg, rank_g] = dd.astype(np.int16)
        esrc[p_of, gg * B + b_of] = mm.astype(np.float32)
        eww[p_of, gg * B + b_of] = ww
        # gathers are split into <=CB-block chunks (SWDGE descriptor scratch
        # holds ~1024 16B descriptors). wrap per chunk: element i of a chunk
        # reads idxs[i%16, i//16]; replicate the 16-row pattern x8.
        gidx = np.zeros((P, G * C2), dtype=np.int16)
        for gi in range(G):
            col = gi * C2
            for off, sz in chunks:
                seg = idx_flat[gi, off * P:(off + sz) * P]
                wr = seg.reshape(sz * P // 16, 16).T       # [16, sz*8]
                gidx[:, col:col + sz * 8] = np.tile(wr, (8, 1))
                col += sz * 8

        lo = c * NPC
        m_own = (d_sorted >= lo) & (d_sorted < lo + NPC)
        d_own = d_sorted[m_own] - lo
        w_own = w_sorted[m_own]
        cnt_own = np.bincount(d_own, minlength=NPC)
        st = np.zeros(NPC + 1, dtype=np.int64)
        st[1:] = np.cumsum(cnt_own)
        rk = np.arange(len(d_own)) - st[d_own]
        wpad_own = np.zeros((P, G * LDo), dtype=np.float32)
        wpad_own[(d_own % P).astype(np.int64), (d_own // P) * LDo + rk] = w_own

        in_maps.append(dict(
            x=x_f32.astype(bf),
            wpad=wpad.astype(bf),
            wpad_own=wpad_own.astype(bf),
            gidx=gidx,
            esrc=esrc.astype(bf),
            eww=eww.astype(bf),
            wmat=wmat_f32,
            iota_rep=iota_rep,
        ))
    return in_maps, (B, LD, LDo)


# --------------------------------------------------------------- bass kernel
def build(B, LD, LDo):
    f32 = mybir.dt.float32
    bf16 = mybir.dt.bfloat16
    i16 = mybir.dt.int16
    AF = mybir.ActivationFunctionType
    OP = mybir.AluOpType
    AX = mybir.AxisListType
    C2 = B * P // 16

    nc = bacc.Bacc("TRN2", debug=False, num_swdge_queues=4)

    x_d = nc.dram_tensor("x", [N, F], bf16, kind="ExternalInput")
    wpad_d = nc.dram_tensor("wpad", [P, NG * LD], bf16, kind="ExternalInput")
    wpado_d = nc.dram_tensor("wpad_own", [P, G * LDo], bf16, kind="ExternalInput")
    gidx_d = nc.dram_tensor("gidx", [P, G * C2], i16, kind="ExternalInput")
    esrc_d = nc.dram_tensor("esrc", [P, G * B], bf16, kind="ExternalInput")
    eww_d = nc.dram_tensor("eww", [P, G * B], bf16, kind="ExternalInput")
    wmat_d = nc.dram_tensor("wmat", [F, F], f32, kind="ExternalInput")
    iota_d = nc.dram_tensor("iota_rep", [P, B * P], bf16, kind="ExternalInput")
    out_d = nc.dram_tensor("out", [NPC, F], f32, kind="ExternalOutput")

    with TileContext(nc) as tc:
        with (
            tc.tile_pool(name="const", bufs=1) as cpool,
            tc.tile_pool(name="pha", bufs=3) as apool,
            tc.tile_pool(name="phb", bufs=3) as bpool,
            tc.tile_pool(name="psum", bufs=2, space="PSUM") as ppool,
            tc.tile_pool(name="psum2", bufs=2, space="PSUM") as ppool2,
            tc.tile_pool(name="dram", bufs=1, space="DRAM") as dpool,
        ):
            from concourse import library_config
            nc.gpsimd.load_library(library_config.mlp)
            # ---- constants ----
            wmat_f = cpool.tile([F, F], f32)
            nc.sync.dma_start(out=wmat_f[:, :], in_=wmat_d[:, :])
            wmat_b = cpool.tile([F, F], bf16)
            nc.vector.tensor_copy(out=wmat_b[:, :], in_=wmat_f[:, :])
            iota_t = cpool.tile([P, B * P], bf16)
            nc.sync.dma_start(out=iota_t[:, :], in_=iota_d[:, :])
            gidx_t = cpool.tile([P, G * C2], i16)
            nc.sync.dma_start(out=gidx_t[:, :], in_=gidx_d[:, :])
            esrc_t = cpool.tile([P, G * B], bf16)
            nc.sync.dma_start(out=esrc_t[:, :], in_=esrc_d[:, :])
            eww_t = cpool.tile([P, G * B], bf16)
            nc.sync.dma_start(out=eww_t[:, :], in_=eww_d[:, :])

            # ---- phase A: deg -> dis; xs = dis*x -> DRAM scratch ----
            deg_t = cpool.tile([P, NG], f32)
            NCH = 4
            CG = NG // NCH
            for t in range(NCH):
                wp = apool.tile([P, CG * LD], bf16, tag="wp")
                nc.sync.dma_start(
                    out=wp[:, :], in_=wpad_d[:, t * CG * LD:(t + 1) * CG * LD])
                nc.vector.tensor_reduce(
                    out=deg_t[:, t * CG:(t + 1) * CG],
                    in_=wp[:, :].rearrange("p (g l) -> p g l", l=LD),
                    axis=AX.X, op=OP.add)
            sqd_t = cpool.tile([P, NG], f32)
            nc.scalar.activation(out=sqd_t[:, :], in_=deg_t[:, :], func=AF.Sqrt)
            dis_t = cpool.tile([P, NG], f32)
            nc.vector.reciprocal(out=dis_t[:, :], in_=sqd_t[:, :])
            disb_t = cpool.tile([P, NG], bf16)
            nc.vector.tensor_copy(out=disb_t[:, :], in_=dis_t[:, :])

            dego_t = cpool.tile([P, G], f32)
            wpo = apool.tile([P, G * LDo], bf16, tag="wpo")
            nc.sync.dma_start(out=wpo[:, :], in_=wpado_d[:, :])
            nc.vector.tensor_reduce(
                out=dego_t[:, :],
                in_=wpo[:, :].rearrange("p (g l) -> p g l", l=LDo),
                axis=AX.X, op=OP.add)
            sqdo_t = cpool.tile([P, G], f32)
            nc.scalar.activation(out=sqdo_t[:, :], in_=dego_t[:, :], func=AF.Sqrt)
            diso_t = cpool.tile([P, G], f32)
            nc.vector.reciprocal(out=diso_t[:, :], in_=sqdo_t[:, :])

            xs_dram = dpool.tile([N, F], bf16)
            XCH = 8
            RG = NG // XCH
            x3 = x_d[:, :].rearrange("(p j) f -> p j f", p=P)
            xs3 = xs_dram[:, :].rearrange("(p j) f -> p j f", p=P)
            for t in range(XCH):
                sl = slice(t * RG, (t + 1) * RG)
                xt = apool.tile([P, RG, F], bf16, tag="xt")
                nc.sync.dma_start(out=xt[:, :, :], in_=x3[:, sl, :])
                xst = apool.tile([P, RG, F], bf16, tag="xst")
                nc.vector.tensor_tensor(
                    out=xst[:, :, :],
                    in0=xt[:, :, :],
                    in1=disb_t[:, sl].to_broadcast([P, RG, F]),
                    op=OP.mult)
                nc.sync.dma_start(out=xs3[:, sl, :], in_=xst[:, :, :])

            # ---- phase B: gather + selector matmuls per 128-src group ----
            chunks = []
            off = 0
            while off < B:
                sz = min(CB, B - off)
                chunks.append((off, sz))
                off += sz
            qn = 0
            for g in range(G):
                xg = bpool.tile([P, B, F], bf16, tag="xg")
                for off, sz in chunks:
                    nc.gpsimd.dma_gather(
                        xg[:, off:off + sz, :],
                        xs_dram[:, :],
                        gidx_t[:, g * C2 + off * 8:g * C2 + (off + sz) * 8],
                        sz * P,
                        sz * P,
                        F,
                        elem_step=F,
                        queue_num=qn % 4,
                    )
                    qn += 1
                sel = bpool.tile([P, B * P], bf16, tag="sel")
                nc.vector.tensor_tensor(
                    out=sel[:, :].rearrange("p (b m) -> p b m", m=P),
                    in0=iota_t[:, :].rearrange("p (b m) -> p b m", m=P),
                    in1=esrc_t[:, g * B:(g + 1) * B].to_broadcast([P, B, P]),
                    op=OP.is_equal)
                xw = bpool.tile([P, B, F], bf16, tag="xw")
                nc.vector.tensor_tensor(
                    out=xw[:, :, :],
                    in0=xg[:, :, :],
                    in1=eww_t[:, g * B:(g + 1) * B].to_broadcast([P, B, F]),
                    op=OP.mult)
                aggp = ppool.tile([F, P], f32, space="PSUM")
                for b in range(B):
                    nc.tensor.matmul(
                        out=aggp[:, :],
                        lhsT=xw[:, b, :],
                        rhs=sel[:, b * P:(b + 1) * P],
                        start=(b == 0),
                        stop=(b == B - 1))
                aggT = bpool.tile([F, P], bf16, tag="aggT")
                nc.vector.tensor_copy(out=aggT[:, :], in_=aggp[:, :])
                outp = ppool2.tile([P, F], f32, space="PSUM")
                nc.tensor.matmul(
                    out=outp[:, :], lhsT=aggT[:, :], rhs=wmat_b[:, :],
                    start=True, stop=True)
                outs = bpool.tile([P, F], f32, tag="outs")
                nc.vector.tensor_scalar(
                    out=outs[:, :], in0=outp[:, :],
                    scalar1=diso_t[:, g:g + 1], scalar2=None, op0=OP.mult)
                nc.sync.dma_start(
                    out=out_d[g * P:(g + 1) * P, :], in_=outs[:, :])

    nc.compile()
    return nc


_NC_CACHE = {}


def run(inputs, trace=False, trace_kwargs=None):
    in_maps, shape_key = prep(
        inputs["x"], inputs["edge_index"], inputs["edge_weight"],
        inputs["weight"])
    if shape_key not in _NC_CACHE:
        _NC_CACHE[shape_key] = build(*shape_key)
    nc = _NC_CACHE[shape_key]
    res = run_bass_kernel_spmd(
        nc, in_maps, core_ids=list(range(NC_CORES)),
        trace=trace, **(trace_kwargs or {}))
    out = np.concatenate([r["out"] for r in res.results], axis=0)
    return out.astype(np.float32), res


def kernel(**inputs):
    out, _ = run(inputs)
    return out


if __name__ == "__main__":
    pass


# revision 17
# speedup vs baseline: 1.2604x; 1.0414x over previous
"""GCN layer (dense-adj reference semantics) as a Bass/Tile kernel on 8
Trainium2 NeuronCores.

Strategy: edge-partitioned sparse message passing instead of the dense
16384x16384 adjacency.
  out[i] = dis[i] * ( sum_{e:(i,j)} w_e * dis[j] * x[j] + dis[i]... ) @ W
computed as
  xs = dis (.) x                (node scaling, bf16)
  aggT[f, i] = sum_e w_e xs[dst_e, f]   (gather + selector matmuls on PE)
  out = dis (.) (aggT^T @ W)
with deg[j] = 1 + sum_{e: dst=j} w_e reduced on-device from a host-packed
padded layout. Src nodes are row-sharded across the 8 cores; each core
processes only its own edges (grouped by 128-src groups, 128-edge blocks).
Self loops (the +I term) are injected as synthetic edges with weight 1.
Duplicate (src,dst) edges resolve last-wins, matching XLA scatter-set.
"""
import numpy as np
import ml_dtypes

import concourse.bass as bass
import concourse.bacc as bacc
import concourse.mybir as mybir
from concourse.tile import TileContext
from concourse.bass_utils import run_bass_kernel_spmd

P = 128
NC_CORES = 8
N = 16384
F = 128
NPC = N // NC_CORES    # 2048 src rows per core
G = NPC // P           # 16 src groups per core
NG = N // P            # 128 node groups
CB = 7                 # gather chunk size in 128-edge blocks (desc scratch)


# ----------------------------------------------------------------- host prep
def _dedup_last(src, dst, ew):
    key = src.astype(np.int64) * N + dst.astype(np.int64)
    order = np.argsort(key, kind="stable")
    ks = key[order]
    run_last = np.concatenate([ks[1:] != ks[:-1], [True]])
    keep = order[run_last]
    return src[keep], dst[keep], ew[keep]


def prep(x, edge_index, edge_weight, weight):
    src = np.asarray(edge_index[0]).astype(np.int64)
    dst = np.asarray(edge_index[1]).astype(np.int64)
    ew = np.asarray(edge_weight).astype(np.float32)
    s, d, w = _dedup_last(src, dst, ew)
    # synthetic self edges = the +eye term
    s = np.concatenate([s, np.arange(N, dtype=np.int64)])
    d = np.concatenate([d, np.arange(N, dtype=np.int64)])
    w = np.concatenate([w, np.ones(N, dtype=np.float32)])
    E2 = len(s)

    # deg inputs packed [p = j%128, col = (j//128)*LD + rank], zero padded
    order_d = np.argsort(d, kind="stable")
    d_sorted = d[order_d]
    w_sorted = w[order_d]
    cnt_in = np.bincount(d_sorted, minlength=N)
    LD = int(cnt_in.max())
    starts = np.zeros(N + 1, dtype=np.int64)
    starts[1:] = np.cumsum(cnt_in)
    rank = np.arange(E2) - starts[d_sorted]
    wpad = np.zeros((P, NG * LD), dtype=np.float32)
    wpad[(d_sorted // P).astype(np.int64), (d_sorted % P) * LD + rank] = w_sorted

    core = s // NPC
    per_core = []
    B = 0
    for c in range(NC_CORES):
        m_c = core == c
        sc, dc, wc = s[m_c], d[m_c], w[m_c]
        sl = sc - c * NPC
        g = sl // P
        m = sl % P
        cnt_g = np.bincount(g, minlength=G)
        B = max(B, int(np.ceil(cnt_g.max() / P)))
        per_core.append((g, m, dc, wc, cnt_g))

    LDo = 0
    for c in range(NC_CORES):
        lo, hi = c * NPC, (c + 1) * NPC
        m_own = (d_sorted >= lo) & (d_sorted < hi)
        LDo = max(LDo, int(np.bincount(d_sorted[m_own] - lo, minlength=NPC).max()))

    C2 = B * P // 16
    chunks = []
    off = 0
    while off < B:
        sz = min(CB, B - off)
        chunks.append((off, sz))
        off += sz
    bf = ml_dtypes.bfloat16
    x_f32 = np.ascontiguousarray(np.asarray(x).astype(np.float32))
    wmat_f32 = np.ascontiguousarray(np.asarray(weight).astype(np.float32))
    iota_rep = np.tile(np.arange(P, dtype=np.float32), (P, B)).astype(bf)

    in_maps = []
    for c in range(NC_CORES):
        g, m, dc, wc, cnt_g = per_core[c]
        order_g = np.argsort(g, kind="stable")
        gg, mm, dd, ww = g[order_g], m[order_g], dc[order_g], wc[order_g]
        starts_g = np.zeros(G + 1, dtype=np.int64)
        starts_g[1:] = np.cumsum(cnt_g)
        rank_g = np.arange(len(gg)) - starts_g[gg]
        b_of = rank_g // P
        p_of = rank_g % P

        idx_flat = np.zeros((G, B * P), dtype=np.int16)
        esrc = np.zeros((P, G * B), dtype=np.float32)
        eww = np.zeros((P, G * B), dtype=np.float32)
        idx_flat[gg, rank_g] = dd.astype(np.int16)
        esrc[p_of, gg * B + b_of] = mm.astype(np.float32)
        eww[p_of, gg * B + b_of] = ww
        # gathers are split into <=CB-block chunks (SWDGE descriptor scratch
        # holds ~1024 16B descriptors). wrap per chunk: element i of a chunk
        # reads idxs[i%16, i//16]; replicate the 16-row pattern x8.
        gidx = np.zeros((P, G * C2), dtype=np.int16)
        for gi in range(G):
            col = gi * C2
            for off, sz in chunks:
                seg = idx_flat[gi, off * P:(off + sz) * P]
                wr = seg.reshape(sz * P // 16, 16).T       # [16, sz*8]
                gidx[:, col:col + sz * 8] = np.tile(wr, (8, 1))
                col += sz * 8

        lo = c * NPC
        m_own = (d_sorted >= lo) & (d_sorted < lo + NPC)
        d_own = d_sorted[m_own] - lo
        w_own = w_sorted[m_own]
        cnt_own = np.bincount(d_own, minlength=NPC)
        st = np.zeros(NPC + 1, dtype=np.int64)
        st[1:] = np.cumsum(cnt_own)
        rk = np.arange(len(d_own)) - st[d_own]
        wpad_own = np.zeros((P, G * LDo), dtype=np.float32)
        wpad_own[(d_own % P).astype(np.int64), (d_own // P) * LDo + rk] = w_own

        in_maps.append(dict(
            x=x_f32.astype(bf),
            wpad=wpad.astype(bf),
            wpad_own=wpad_own.astype(bf),
            gidx=gidx,
            esrc=esrc.astype(bf),
            eww=eww.astype(bf),
            wmat=wmat_f32,
            iota_rep=iota_rep,
        ))
    return in_maps, (B, LD, LDo)


# --------------------------------------------------------------- bass kernel
def build(B, LD, LDo):
    f32 = mybir.dt.float32
    bf16 = mybir.dt.bfloat16
    i16 = mybir.dt.int16
    AF = mybir.ActivationFunctionType
    OP = mybir.AluOpType
    AX = mybir.AxisListType
    C2 = B * P // 16

    nc = bacc.Bacc("TRN2", debug=False, num_swdge_queues=4)

    x_d = nc.dram_tensor("x", [N, F], bf16, kind="ExternalInput")
    wpad_d = nc.dram_tensor("wpad", [P, NG * LD], bf16, kind="ExternalInput")
    wpado_d = nc.dram_tensor("wpad_own", [P, G * LDo], bf16, kind="ExternalInput")
    gidx_d = nc.dram_tensor("gidx", [P, G * C2], i16, kind="ExternalInput")
    esrc_d = nc.dram_tensor("esrc", [P, G * B], bf16, kind="ExternalInput")
    eww_d = nc.dram_tensor("eww", [P, G * B], bf16, kind="ExternalInput")
    wmat_d = nc.dram_tensor("wmat", [F, F], f32, kind="ExternalInput")
    iota_d = nc.dram_tensor("iota_rep", [P, B * P], bf16, kind="ExternalInput")
    out_d = nc.dram_tensor("out", [NPC, F], f32, kind="ExternalOutput")

    with TileContext(nc) as tc:
        with (
            tc.tile_pool(name="const", bufs=1) as cpool,
            tc.tile_pool(name="pha", bufs=3) as apool,
            tc.tile_pool(name="phb", bufs=4) as bpool,
            tc.tile_pool(name="psum", bufs=2, space="PSUM") as ppool,
            tc.tile_pool(name="psum2", bufs=2, space="PSUM") as ppool2,
            tc.tile_pool(name="dram", bufs=1, space="DRAM") as dpool,
        ):
            from concourse import library_config
            nc.gpsimd.load_library(library_config.mlp)
            # ---- constants ----
            wmat_f = cpool.tile([F, F], f32)
            nc.sync.dma_start(out=wmat_f[:, :], in_=wmat_d[:, :])
            wmat_b = cpool.tile([F, F], bf16)
            nc.vector.tensor_copy(out=wmat_b[:, :], in_=wmat_f[:, :])
            iota_t = cpool.tile([P, B * P], bf16)
            nc.sync.dma_start(out=iota_t[:, :], in_=iota_d[:, :])
            gidx_t = cpool.tile([P, G * C2], i16)
            nc.sync.dma_start(out=gidx_t[:, :], in_=gidx_d[:, :])
            esrc_t = cpool.tile([P, G * B], bf16)
            nc.sync.dma_start(out=esrc_t[:, :], in_=esrc_d[:, :])
            eww_t = cpool.tile([P, G * B], bf16)
            nc.sync.dma_start(out=eww_t[:, :], in_=eww_d[:, :])

            # ---- phase A: deg -> dis; xs = dis*x -> DRAM scratch ----
            deg_t = cpool.tile([P, NG], f32)
            NCH = 4
            CG = NG // NCH
            for t in range(NCH):
                wp = apool.tile([P, CG * LD], bf16, tag="wp")
                nc.sync.dma_start(
                    out=wp[:, :], in_=wpad_d[:, t * CG * LD:(t + 1) * CG * LD])
                nc.vector.tensor_reduce(
                    out=deg_t[:, t * CG:(t + 1) * CG],
                    in_=wp[:, :].rearrange("p (g l) -> p g l", l=LD),
                    axis=AX.X, op=OP.add)
            sqd_t = cpool.tile([P, NG], f32)
            nc.scalar.activation(out=sqd_t[:, :], in_=deg_t[:, :], func=AF.Sqrt)
            dis_t = cpool.tile([P, NG], f32)
            nc.vector.reciprocal(out=dis_t[:, :], in_=sqd_t[:, :])
            disb_t = cpool.tile([P, NG], bf16)
            nc.vector.tensor_copy(out=disb_t[:, :], in_=dis_t[:, :])

            dego_t = cpool.tile([P, G], f32)
            wpo = apool.tile([P, G * LDo], bf16, tag="wpo")
            nc.sync.dma_start(out=wpo[:, :], in_=wpado_d[:, :])
            nc.vector.tensor_reduce(
                out=dego_t[:, :],
                in_=wpo[:, :].rearrange("p (g l) -> p g l", l=LDo),
                axis=AX.X, op=OP.add)
            sqdo_t = cpool.tile([P, G], f32)
            nc.scalar.activation(out=sqdo_t[:, :], in_=dego_t[:, :], func=AF.Sqrt)
            diso_t = cpool.tile([P, G], f32)
            nc.vector.reciprocal(out=diso_t[:, :], in_=sqdo_t[:, :])

            xs_dram = dpool.tile([N, F], bf16)
            XCH = 8
            RG = NG // XCH
            x3 = x_d[:, :].rearrange("(p j) f -> p j f", p=P)
            xs3 = xs_dram[:, :].rearrange("(p j) f -> p j f", p=P)
            for t in range(XCH):
                sl = slice(t * RG, (t + 1) * RG)
                xt = apool.tile([P, RG, F], bf16, tag="xt")
                nc.sync.dma_start(out=xt[:, :, :], in_=x3[:, sl, :])
                xst = apool.tile([P, RG, F], bf16, tag="xst")
                nc.vector.tensor_tensor(
                    out=xst[:, :, :],
                    in0=xt[:, :, :],
                    in1=disb_t[:, sl].to_broadcast([P, RG, F]),
                    op=OP.mult)
                nc.sync.dma_start(out=xs3[:, sl, :], in_=xst[:, :, :])

            # ---- phase B: gather + selector matmuls per 128-src group ----
            chunks = []
            off = 0
            while off < B:
                sz = min(CB, B - off)
                chunks.append((off, sz))
                off += sz
            qn = 0
            for g in range(G):
                xg = bpool.tile([P, B, F], bf16, tag="xg")
                for off, sz in chunks:
                    nc.gpsimd.dma_gather(
                        xg[:, off:off + sz, :],
                        xs_dram[:, :],
                        gidx_t[:, g * C2 + off * 8:g * C2 + (off + sz) * 8],
                        sz * P,
                        sz * P,
                        F,
                        elem_step=F,
                        queue_num=qn % 4,
                    )
                    qn += 1
                sel = bpool.tile([P, B * P], bf16, tag="sel")
                nc.vector.tensor_tensor(
                    out=sel[:, :].rearrange("p (b m) -> p b m", m=P),
                    in0=iota_t[:, :].rearrange("p (b m) -> p b m", m=P),
                    in1=esrc_t[:, g * B:(g + 1) * B].to_broadcast([P, B, P]),
                    op=OP.is_equal)
                xw = bpool.tile([P, B, F], bf16, tag="xw")
                nc.vector.tensor_tensor(
                    out=xw[:, :, :],
                    in0=xg[:, :, :],
                    in1=eww_t[:, g * B:(g + 1) * B].to_broadcast([P, B, F]),
                    op=OP.mult)
                aggp = ppool.tile([F, P], f32, space="PSUM")
                for b in range(B):
                    nc.tensor.matmul(
                        out=aggp[:, :],
                        lhsT=xw[:, b, :],
                        rhs=sel[:, b * P:(b + 1) * P],
                        start=(b == 0),
                        stop=(b == B - 1))
                aggT = bpool.tile([F, P], bf16, tag="aggT")
                nc.vector.tensor_copy(out=aggT[:, :], in_=aggp[:, :])
                outp = ppool2.tile([P, F], f32, space="PSUM")
                nc.tensor.matmul(
                    out=outp[:, :], lhsT=aggT[:, :], rhs=wmat_b[:, :],
                    start=True, stop=True)
                outs = bpool.tile([P, F], f32, tag="outs")
                nc.vector.tensor_scalar(
                    out=outs[:, :], in0=outp[:, :],
                    scalar1=diso_t[:, g:g + 1], scalar2=None, op0=OP.mult)
                nc.sync.dma_start(
                    out=out_d[g * P:(g + 1) * P, :], in_=outs[:, :])

    nc.compile()
    return nc


_NC_CACHE = {}


def run(inputs, trace=False, trace_kwargs=None):
    in_maps, shape_key = prep(
        inputs["x"], inputs["edge_index"], inputs["edge_weight"],
        inputs["weight"])
    if shape_key not in _NC_CACHE:
        _NC_CACHE[shape_key] = build(*shape_key)
    nc = _NC_CACHE[shape_key]
    res = run_bass_kernel_spmd(
        nc, in_maps, core_ids=list(range(NC_CORES)),
        trace=trace, **(trace_kwargs or {}))
    out = np.concatenate([r["out"] for r in res.results], axis=0)
    return out.astype(np.float32), res


def kernel(**inputs):
    out, _ = run(inputs)
    return out


if __name__ == "__main__":
    pass
